# revision 87
# baseline (speedup 1.0000x reference)
"""GQA attention kernel for 8 TRN2 NeuronCores — chunk-pipelined v3.

Problem: B=2, T=2048, C=4096, NH=32 q-heads, NKV=8 kv-heads, HD=128,
RoPE (theta=1e4), causal, f32 I/O.

Sharding: core = (batch b, head-group g): b = core//4, g = core%4.
Each core owns batch b, kv heads {2g, 2g+1} (= q heads 8g..8g+7).

v3 changes over v2 (which was PE-busy 94.7% with ~39us of gaps; v3
measures ~16us of gaps, ~724-732us):
- DRAM layouts are partition-major within each DMA tile (x pieces,
  wqkv, wo, out) so consecutive partitions are contiguous and the DGE
  aggregates descriptors (~2x effective queue throughput).
- Queue discipline, learned the hard way: the weight stream owns the
  sync HWDGE queue exclusively — any XBAR transpose interleaved into a
  streaming queue degrades BOTH to ~2us per 2KB descriptor; a deep
  reuse-gated stream on the scalar HWDGE queue back-pressures the ACT
  engine itself (blocked pushing descriptors). So: weights = sync; V
  and bulk-OT transposes = scalar, issued only where ACT idles
  (projection windows); x prefetch + attention-phase wo loads = gpsimd
  SWDGE; drain-phase wo loads = sync.
- o_proj is chunk-pipelined into chunk-3's attention: chunk 3's
  attention is ACT(exp)-bound, so o_proj matmul packets for tokens
  0..1535 (final after chunk 2) interleave between heads to keep the
  PE fed, then drain after. OT and the feature-major OTT are both
  persistent: OTT allocated from released-pool space would inherit an
  SBUF-reuse dependency on the last projection matmul and miss the
  packets. The tail o_proj pass (tokens 1536+) uses per-head sync
  transposes emitted right after each AV.
- scores(h+1) are emitted before AV(h) (one-head lookahead, chunks
  0-2) and the last head's AV defers into the next chunk's projection
  stream, so the PE never sits behind exp latency at chunk tails.
  Chunk 3 uses packets instead of lookahead (12 ptile slots would
  deadlock on 2x8 groups).
- exp of the last diagonal score group is trimmed to its causal
  columns (384 of 1024); the causal mask is a single shared [128,128]
  upper-triangular block (identical for every diagonal tile).
- warm-up uses junk matmuls on a memset tile (no ident DMA dependency,
  no transpose->matmul PE array mode switch); chunk-0 x lands split
  across the sync+scalar queues ahead of the weight stream.
- o_proj accumulates all 8 heads into one PSUM bank (no psA/psB split,
  halving the DVE copy traffic); partial outputs ship as bf16 (adds
  ~1e-4 to rel err; total 6.3e-3 vs gate 2e-2).

RoPE rotate_half runs on the DVE as a 32-lane quadrant shuffle: q/k
head features are permuted on the host (scores are invariant under a
shared q/k permutation) so each rope pair sits 16 partitions apart
within a quadrant, and the sign folds into the sin table.

fp8 (DoubleRow, 2x matmul throughput) was evaluated and is accuracy-
infeasible: e4m3 quantization of any single GEMM stage adds 3.8-6e-2
rel err vs the 2e-2 gate (measured in numpy simulation).
"""

import sys

sys.path.insert(0, "/opt/trn_rl_repo")

import numpy as np
import ml_dtypes

import concourse.bass as bass
import concourse.bacc as bacc
import concourse.mybir as mybir
import concourse.tile as tile
from concourse.bass_utils import run_bass_kernel_spmd

BF16 = mybir.dt.bfloat16
F32 = mybir.dt.float32
AF = mybir.ActivationFunctionType
ALU = mybir.AluOpType

B, T, C = 2, 2048, 4096
NH, NKV, HD = 32, 8, 128
THETA = 10000.0
NCORES = 8

QH = 8          # q heads per core
KV = 2          # kv heads per core
QC = 4          # token chunks of 512
CCH = 32        # contraction chunks of 128 over C
NOUT = 12       # projection out tiles per chunk: k0,k1,v0,v1,q0..q7

ROT_MASK = [(i + 16) % 32 for i in range(32)]

_CACHE = {}


def _build_nc():
    nc = bacc.Bacc("TRN2", target_bir_lowering=False, debug=False,
                   enable_asserts=False, num_devices=NCORES)

    # layouts chosen so each DMA's consecutive partitions are contiguous in
    # DRAM: the DGE then aggregates descriptors (~2x queue throughput)
    xT_d = nc.dram_tensor("xT", [QC, 8, 128, 4, 512], BF16,
                          kind="ExternalInput")
    wqkv_d = nc.dram_tensor("wqkv", [NOUT, 4, 128, 2, 512], BF16,
                            kind="ExternalInput")
    wo_d = nc.dram_tensor("wo", [8, 128, QH, 512], BF16,
                          kind="ExternalInput")
    cos_d = nc.dram_tensor("cosT", [128, T], BF16, kind="ExternalInput")
    sin_d = nc.dram_tensor("sinT", [128, T], BF16, kind="ExternalInput")
    cmask_d = nc.dram_tensor("cmask", [128, 128], BF16,
                             kind="ExternalInput")
    out_d = nc.dram_tensor("out", [8, 16, 128, 512], BF16,
                           kind="ExternalOutput")

    with tile.TileContext(nc) as tc:
        with tc.tile_pool(name="persist", bufs=1) as pp:
            warm = pp.tile([128, 128], BF16)
            nc.vector.memset(warm, 0.125)
            cosT = pp.tile([128, T], BF16)
            sinT = pp.tile([128, T], BF16)
            # single upper-triangular [k,q] mask: the diagonal 128-block is
            # the same for every diagonal tile
            cmask = pp.tile([128, 128], BF16)
            nc.gpsimd.dma_start(cosT, cos_d.ap())
            nc.gpsimd.dma_start(sinT, sin_d.ap())
            nc.gpsimd.dma_start(cmask, cmask_d.ap())

            KTt = pp.tile([128, KV, T], BF16)
            QT = pp.tile([128, QH, 512], BF16)      # current chunk only
            OT = pp.tile([128, QH, T], BF16)
            # feature-major attn output for o_proj; persistent so its
            # writes carry no SBUF-reuse dependency on released pools
            OTT = pp.tile([128, QH, 16, 128], BF16)
            Vn = pp.tile([128, KV, 16, 132], BF16)
            nc.vector.memset(Vn[:, :, :, 128:129], 1.0)



            with tc.tile_pool(name="ptp", bufs=12) as ptp, \
                 tc.tile_pool(name="rcp", bufs=2) as rcp, \
                 tc.tile_pool(name="pst", bufs=2, space="PSUM") as stp, \
                 tc.tile_pool(name="ppo", bufs=2, space="PSUM") as pop:

                pj = tc.alloc_tile_pool(name="pproj", bufs=2, space="PSUM")
                xp = tc.alloc_tile_pool(name="xp", bufs=2)
                wtp = tc.alloc_tile_pool(name="wtp", bufs=8)
                vtp = tc.alloc_tile_pool(name="vtp", bufs=1)
                rtp = tc.alloc_tile_pool(name="rtmp", bufs=1)
                rsp = tc.alloc_tile_pool(name="rsp", bufs=1)

                # PE warm-up: junk matmuls (DMA-free) keep the array busy
                # and ramp the p-state while the first x/w DMAs land
                for w in range(100):
                    wps = pj.tile([128, 128], F32, name=f"warm{w}", tag="pj")
                    nc.tensor.matmul(wps, warm, warm, start=True, stop=True)

                def load_x(qc, queues):
                    # 8 pieces of 4 cc each; queues[pc] picks the DMA queue
                    xt = xp.tile([128, CCH, 512], BF16, name=f"xt{qc}",
                                 tag="xt")
                    for pc in range(8):
                        queues[pc].dma_start(xt[:, 4 * pc:4 * pc + 4, :],
                                             xT_d.ap()[qc, pc])
                    return xt

                xts = [None] * QC
                # chunk 0 split across the two fast HWDGE queues: a small
                # sync share (so the weight stream starts early behind it),
                # the bulk on the otherwise-idle scalar queue
                xts[0] = load_x(0, [nc.sync, nc.sync, nc.sync, nc.scalar,
                                    nc.scalar, nc.scalar, nc.scalar,
                                    nc.scalar])

                # init st slots so stale-region exp stays finite
                for i in range(2):
                    sti = stp.tile([128, 2, 512], F32, name=f"sti{i}",
                                   tag="st")
                    nc.vector.memset(sti, 0.0)

                def rope_write(dst, ps, tsl):
                    # dst = ps*cos + quadshuffle(ps)*sin  (sign baked in sin)
                    tmp = rtp.tile([128, 512], F32)
                    nc.vector.stream_shuffle(tmp, ps, ROT_MASK)
                    rs = rsp.tile([128, 512], BF16)
                    nc.vector.tensor_tensor(rs, tmp, sinT[:, tsl],
                                            op=ALU.mult)
                    nc.vector.tensor_tensor(dst, ps, cosT[:, tsl],
                                            op=ALU.mult)
                    nc.vector.tensor_tensor(dst, dst, rs, op=ALU.add)

                # ---- o_proj packet machinery (created before chunk 3) ----
                opst = {}

                def emit_wo_load(n, pass_id):
                    key = (n, pass_id)
                    wt = opst["wop"].tile([128, QH, 512], BF16,
                                          name=f"wo{pass_id}_{n}", tag="wo")
                    # gpsimd while sync/scalar are busy with attention-phase
                    # traffic, sync (faster) once the drain starts
                    q = nc.gpsimd if opst["in_attn"] else nc.sync
                    q.dma_start(wt, wo_d.ap()[n])
                    opst[key] = wt

                def emit_packet():
                    # one (n, tt) o_proj output block: 8-head accumulation
                    idx = opst["idx"]
                    if idx >= 128:
                        return False
                    opst["idx"] = idx + 1
                    pass_id, rem = divmod(idx, 96)
                    if pass_id == 0:
                        n, tt = divmod(rem, 12)
                    else:
                        n, tt = divmod(rem, 4)
                        tt += 12
                    if tt % (12 if pass_id == 0 else 4) == 0:
                        # starting a new n: prefetch the next wo slice
                        nxt = n + 2
                        if pass_id == 0 and nxt < 8:
                            emit_wo_load(nxt, 0)
                        elif pass_id == 0:
                            emit_wo_load(nxt - 8, 1)
                        elif nxt < 8:
                            emit_wo_load(nxt, 1)
                    wo_t = opst[(n, pass_id)]
                    ps = opst["opj"].tile([128, 512], F32, tag="op")
                    for h in range(QH):
                        nc.tensor.matmul(ps, OTT[:, h, tt, :],
                                         wo_t[:, h, :],
                                         start=(h == 0), stop=(h == QH - 1))
                    stg = opst["stgp"].tile([128, 512], BF16)
                    nc.vector.tensor_copy(stg, ps)
                    q = nc.sync if (idx % 2 == 0 or opst["in_attn"]) \
                        else nc.scalar
                    q.dma_start(out_d.ap()[n, tt], stg)
                    return True

                for qc in range(QC):
                    tsl = slice(qc * 512, (qc + 1) * 512)
                    qcb = qc % 2

                    # prefetch next chunk's x on the gpsimd SWDGE queue
                    if qc + 1 < QC:
                        xts[qc + 1] = load_x(qc + 1, [nc.gpsimd] * 8)
                    xt = xts[qc]

                    # ---- projections for chunk qc ----
                    for o in range(NOUT):
                        ps = pj.tile([128, 512], F32, name=f"pj{qc}_{o}",
                                     tag="pj")
                        for cc in range(4):
                            wt = wtp.tile([128, 2, 512], BF16)
                            # the whole weight stream stays on sync: XBAR
                            # transposes or reuse-gated loads sharing the
                            # queue trickle it, and a deep stream on the
                            # scalar HWDGE queue blocks the ACT engine
                            nc.sync.dma_start(wt, wqkv_d.ap()[o, cc])
                            for k in range(8):
                                c = cc * 8 + k
                                nc.tensor.matmul(
                                    ps,
                                    wt[:, k // 4, (k % 4) * 128:
                                       (k % 4 + 1) * 128],
                                    xt[:, c, :],
                                    start=(c == 0), stop=(c == CCH - 1))
                        if o == 2 and "pend_av" in opst:
                            # previous chunk's last-head AV, deferred past
                            # this o-group so its exps drain under real PE
                            # work instead of stalling the attention tail
                            f, a = opst.pop("pend_av")
                            f(*a)
                        if o < 2:
                            rope_write(KTt[:, o, tsl], ps, tsl)
                        elif o >= 4:
                            h = o - 4
                            rope_write(QT[:, h, :], ps, tsl)
                        else:
                            kvi = o - 2
                            vt = vtp.tile([128, 512], BF16)
                            nc.vector.tensor_copy(vt, ps)
                            # blocked XBAR transpose: [hd,512] -> 4x[tok,hd]
                            # (contiguous staging: strided transpose targets
                            # are silently mis-written by the XBAR path)
                            # on scalar: XBAR transposes must never share
                            # a queue with the weight stream (the mixed
                            # descriptor feed trickles both to ~2us/2KB)
                            vc = vtp.tile([128, 4, 128], BF16, tag="vc")
                            nc.scalar.dma_start_transpose(vc, vt)
                            nc.vector.tensor_copy(
                                Vn[:, kvi, qc * 4:(qc + 1) * 4, 0:128], vc)

                    if qc == 3:
                        # last projections emitted: free the projection-phase
                        # pools (LIFO) and stand up the o_proj machinery so
                        # packets can interleave with chunk-3 attention
                        rsp.release()
                        rtp.release()
                        vtp.release()
                        wtp.release()
                        xp.release()
                        pj.release()
                        opst["wop"] = tc.alloc_tile_pool(name="wop", bufs=3)
                        opst["stgp"] = tc.alloc_tile_pool(name="stgp", bufs=8)
                        opst["opj"] = tc.alloc_tile_pool(
                            name="popj", bufs=2, space="PSUM")
                        opst["idx"] = 0
                        opst["in_attn"] = True
                        emit_wo_load(0, 0)
                        emit_wo_load(1, 0)
                        # bulk feature-major transpose of tokens 0..1535
                        # (final since chunk-2's attention) on the scalar
                        # queue: empty here, and the ACT issue cost lands
                        # in this projection window where ACT idles. OTT
                        # is persistent so no SBUF-reuse wait delays this.
                        for h in range(QH):
                            nc.scalar.dma_start_transpose(
                                OTT[:, h, 0:12, :], OT[:, h, 0:1536])

                    # ---- attention for chunk qc, one-head lookahead ----
                    NG = 2 * qc + 2


                    def emit_scores(h):
                        kv = h // 4
                        pts = []
                        for g in range(NG):
                            st = stp.tile([128, 2, 512], F32, tag="st")
                            for i in range(2):
                                kt = 2 * g + i
                                d = kt - 4 * qc
                                ksl = slice(kt * 128, (kt + 1) * 128)
                                if d < 0:
                                    nc.tensor.matmul(
                                        st[:, i, :], KTt[:, kv, ksl],
                                        QT[:, h, :],
                                        start=True, stop=True)
                                else:
                                    nc.tensor.matmul(
                                        st[:, i, d * 128:],
                                        KTt[:, kv, ksl],
                                        QT[:, h, d * 128:],
                                        start=True, stop=True)
                            if g >= NG - 2:
                                # only the diagonal-straddling 128-block
                                # needs masking; the below-block region is
                                # stale psum that AV never reads
                                for i in range(2):
                                    d = 2 * g + i - 4 * qc
                                    bsl = slice(d * 128, (d + 1) * 128)
                                    nc.vector.tensor_tensor(
                                        st[:, i, bsl], st[:, i, bsl],
                                        cmask, op=ALU.add)
                            ptile = ptp.tile([128, 2, 512], BF16)
                            if g == NG - 1:
                                # trim exp to the causal columns (AV never
                                # reads the rest): 384 of 1024 columns
                                nc.scalar.activation(ptile[:, 0, 256:],
                                                     st[:, 0, 256:], AF.Exp)
                                nc.scalar.activation(ptile[:, 1, 384:],
                                                     st[:, 1, 384:], AF.Exp)
                            else:
                                nc.scalar.activation(ptile, st, AF.Exp)
                            pts.append(ptile)
                        return pts

                    def emit_av(h, pts, qc=qc):
                        # qc bound at def time: the last head's AV is
                        # deferred into the next chunk's iteration
                        kv = h // 4
                        for j in range(4):
                            qt = 4 * qc + j
                            po = pop.tile([128, 129], F32, tag="po")
                            for kt in range(qt + 1):
                                nc.tensor.matmul(
                                    po,
                                    pts[kt // 2][:, kt % 2,
                                                 j * 128:(j + 1) * 128],
                                    Vn[:, kv, kt, 0:129],
                                    start=(kt == 0), stop=(kt == qt))
                            rc = rcp.tile([128, 1], F32)
                            nc.vector.reciprocal(rc, po[:, 128:129])
                            nc.vector.tensor_scalar_mul(
                                OT[:, h, qt * 128:(qt + 1) * 128],
                                po[:, 0:128], rc)
                        if qc == 3:
                            # this head's last-chunk row of OT is final:
                            # transpose for the tail o_proj pass (sync
                            # carries only small packet stores here)
                            nc.sync.dma_start_transpose(
                                OTT[:, h, 12:16, :], OT[:, h, 1536:2048])

                    if qc == 3:
                        # no lookahead here (ptp has 12 slots, two full
                        # heads of chunk-3 pts would need 16 and deadlock);
                        # o_proj packets cover the exp latency instead
                        for h in range(QH):
                            pts = emit_scores(h)
                            if h >= 1:
                                emit_packet()
                                emit_packet()
                            emit_av(h, pts)
                    else:
                        prev = None
                        for h in range(QH):
                            pts = emit_scores(h)
                            if prev is not None:
                                emit_av(*prev)
                            prev = (h, pts)
                        # defer the last head's AV into the next chunk's
                        # projection stream
                        opst["pend_av"] = (emit_av, prev)

                # ------------- o_proj: drain remaining packets -------------
                opst["in_attn"] = False
                while emit_packet():
                    pass
                opst["opj"].release()
                opst["stgp"].release()
                opst["wop"].release()

    nc.compile()
    return nc


def _host_prep(x, wq, wk, wv, wo):
    bf = ml_dtypes.bfloat16
    scale = HD ** -0.5

    # feature permutation putting rope pairs 16 partitions apart
    perm = np.zeros(128, np.int64)
    for s in range(4):
        for i in range(32):
            perm[32 * s + i] = 16 * s + i if i < 16 else 64 + 16 * s + (i - 16)
    sign = np.array([-1.0 if (i % 32) < 16 else 1.0 for i in range(128)],
                    np.float32)

    inv_freq = 1.0 / (THETA ** (np.arange(0, HD, 2, dtype=np.float32) / HD))
    t = np.arange(T, dtype=np.float32)
    freqs = np.outer(t, inv_freq)                      # [T, 64]
    emb = np.concatenate([freqs, freqs], -1)           # [T, 128]
    cosT = np.ascontiguousarray(np.cos(emb)[:, perm].T).astype(bf)
    sinT = np.ascontiguousarray(
        np.sin(emb)[:, perm].T * sign[:, None]).astype(bf)

    # additive causal mask for the diagonal [128k, 128q] block (identical
    # for every diagonal tile)
    kl = np.arange(128)[:, None]
    ql = np.arange(128)[None, :]
    cmask = np.ascontiguousarray(
        np.where(ql >= kl, 0.0, -1e9).astype(np.float32)).astype(bf)

    xT = []
    for b in range(B):
        # [qc, piece, 128 p, 4 cc, t]: partition-major within each piece so
        # the DGE aggregates descriptors across partitions
        x7 = x[b].reshape(QC, 512, 8, 4, 128).transpose(0, 2, 4, 3, 1)
        xT.append(np.ascontiguousarray(x7).astype(bf))

    def wtile(col, permute):
        # [C, 128] -> [8, 128, 512] (cc, part, k*128+f)
        if permute:
            col = col[:, perm]
        r = col.reshape(8, 4, 128, 128).transpose(0, 2, 1, 3)
        return r.reshape(8, 128, 512)

    wqkv, wob = [], []
    for g in range(4):
        tiles = []
        for kvi in range(2):
            tiles.append(wtile(
                wk[:, g * 256 + kvi * 128: g * 256 + (kvi + 1) * 128], True))
        for kvi in range(2):
            tiles.append(wtile(
                wv[:, g * 256 + kvi * 128: g * 256 + (kvi + 1) * 128], False))
        for h in range(8):
            tiles.append(wtile(
                (wq[:, g * 1024 + h * 128: g * 1024 + (h + 1) * 128]
                 * scale), True))
        w5 = np.stack(tiles, 0)                        # [12, 8, 128, 512]
        w5 = w5.reshape(NOUT, 4, 2, 128, 512).transpose(0, 1, 3, 2, 4)
        wqkv.append(np.ascontiguousarray(w5).astype(bf))
        wos = wo[g * 1024:(g + 1) * 1024, :]           # [1024, C]
        wo5 = wos.reshape(QH, 128, 8, 512).transpose(2, 1, 0, 3)
        wob.append(np.ascontiguousarray(wo5).astype(bf))  # [8n, 128, QH, 512]

    in_maps = []
    for core in range(NCORES):
        b, g = core // 4, core % 4
        in_maps.append({
            "xT": xT[b], "wqkv": wqkv[g], "wo": wob[g],
            "cosT": cosT, "sinT": sinT, "cmask": cmask,
        })
    return in_maps


def kernel(x, wq, wk, wv, wo, _trace=False, _tmpdir=None):
    if "nc" not in _CACHE:
        _CACHE["nc"] = _build_nc()
    nc = _CACHE["nc"]

    in_maps = _host_prep(x, wq, wk, wv, wo)
    res = run_bass_kernel_spmd(nc, in_maps, core_ids=list(range(NCORES)),
                               trace=_trace, tmpdir=_tmpdir)
    _CACHE["last_results"] = res

    out = np.zeros((B, T, C), np.float32)
    for core in range(NCORES):
        r = np.asarray(res.results[core]["out"], np.float32)  # [n,tt,p,f]
        out[core // 4] += r.transpose(1, 2, 0, 3).reshape(T, C)
    return out
